# revision 11
# baseline (speedup 1.0000x reference)
"""Trainium2 Bass kernel for the 2-layer heterogeneous GCN encoder
(gene/drug graph). Self-contained: host-side prep (numpy), Bass/Tile
kernel build, SPMD execution on 8 NeuronCores.

Strategy:
 - Destination-row sharding: core c owns genes [2500c, 2500(c+1)) and
   drugs [375c, 375(c+1)) (padded to 2560/384 rows).
 - Per layer: each core computes its row-shard of X@W per edge type
   (fp16 matmul, fp32 psum), scaled by dinv for the normalized types;
   AllGather the per-type message tables (fp16); per-core gather of
   source rows (SWDGE dma_gather), on-chip segment reduction (DVE add
   tree along the free dim, one dst per partition), placement into
   fp32 accumulators via unique-index dma_scatter_add; dense post
   (dinv scale, bias, relu, l2norm, type-sum).
 - GCN norm factorized: D^-1/2(A+I)D^-1/2 XW = dinv*(A_scaled + self),
   so no per-edge multiply is needed.
 - Dropout masks are deterministic (threefry, key 42) and computed on
   host in pure numpy (bit-exact vs jax), pre-applied to layer-0
   inputs and to layer-1 activations before the layer-1 matmul.
"""
import numpy as np

N_CORES = 8

# ---------------------------------------------------------------- threefry ---

def _rotl(x, r):
    return ((x << np.uint32(r)) | (x >> np.uint32(32 - r))).astype(np.uint32)

_ROT = [[13, 15, 26, 6], [17, 29, 16, 24]]

def _threefry_core(keypair, x0, x1):
    x0 = x0.astype(np.uint32).copy()
    x1 = x1.astype(np.uint32).copy()
    ks0, ks1 = np.uint32(keypair[0]), np.uint32(keypair[1])
    ks2 = np.uint32(ks0 ^ ks1 ^ np.uint32(0x1BD11BDA))
    x0 = (x0 + ks0).astype(np.uint32)
    x1 = (x1 + ks1).astype(np.uint32)
    ks = [ks1, ks2, ks0]
    for i in range(5):
        for r in _ROT[i % 2]:
            x0 = (x0 + x1).astype(np.uint32)
            x1 = _rotl(x1, r)
            x1 = (x1 ^ x0).astype(np.uint32)
        x0 = (x0 + ks[i % 3]).astype(np.uint32)
        x1 = (x1 + ks[(i + 1) % 3] + np.uint32(i + 1)).astype(np.uint32)
    return x0, x1

def _tf_key(seed):
    return (np.uint32(seed >> 32), np.uint32(seed & 0xFFFFFFFF))

def _tf_split(k, num):
    hi, lo = _threefry_core(k, np.zeros(num, np.uint32), np.arange(num, dtype=np.uint32))
    return [(hi[i], lo[i]) for i in range(num)]

def _tf_bernoulli(k, p, shape):
    n = int(np.prod(shape))
    assert n < 2**32
    hi, lo = _threefry_core(k, np.zeros(n, np.uint32), np.arange(n, dtype=np.uint32))
    bits = (hi ^ lo)
    fl = ((bits >> np.uint32(9)) | np.uint32(0x3F800000)).view(np.float32) - np.float32(1.0)
    return (fl < np.float32(p)).reshape(shape)

# ------------------------------------------------------------------- config ---

DIMS = dict(n_gene=20000, n_drug=3000, d_gene=1024, d_drug=512, h0=256, h1=128)
P_DROP = 0.2
CALL_SLOTS = 8      # 8*128 = 1024 gather idxs per SWDGE call (desc-ring cap)
ROUND_SLOTS = 32    # slots per reduce round (gather tile free size)
SC_GROUPS = 4       # groups per placement scatter (4*128 = 512 tokens)


def _pad128(n):
    return ((n + 127) // 128) * 128


def _shard_dims(dims):
    g_sh = dims["n_gene"] // N_CORES
    d_sh = dims["n_drug"] // N_CORES
    gp, dp = _pad128(g_sh), _pad128(d_sh)
    if gp == g_sh:
        gp += 128  # need trash rows
    if dp == d_sh:
        dp += 128
    return g_sh, d_sh, gp, dp


# ---------------------------------------------------------------- host prep ---

def _edge_structure(src, dst, n_dst_total, sh, shp, table_row_of_src, zrow):
    """Per-core gather/placement structure for one edge type.

    Returns (D_g_profile (len NG), gidx_percore [8][Ntot] int16,
    pidx_percore [8][NG*128] int16)."""
    NG = shp // 128
    owner = dst // sh
    local = dst % sh
    srows = table_row_of_src(src)

    # per-core per-dst counts and sorted order
    percore = []
    for c in range(N_CORES):
        sel = owner == c
        loc = local[sel]
        sr = srows[sel]
        cnt = np.bincount(loc, minlength=sh)
        order = np.argsort(-cnt, kind="stable")  # real dsts, degree desc
        pi = np.concatenate([order, np.arange(sh, NG * 128)])  # pad dsts at end
        degs = np.concatenate([cnt[order], np.zeros(NG * 128 - sh, np.int64)])
        percore.append((loc, sr, cnt, pi, degs))

    D_g = np.zeros(NG, np.int64)
    for c in range(N_CORES):
        degs = percore[c][4]
        for g in range(NG):
            D_g[g] = max(D_g[g], degs[g * 128:(g + 1) * 128].max())
    # total idx slots: sum over groups of D_g * 128
    tot = int(D_g.sum() * 128)
    tot16 = ((tot + 15) // 16) * 16

    group_base = np.concatenate([[0], np.cumsum(D_g * 128)]).astype(np.int64)

    gidx_all, pidx_all = [], []
    for c in range(N_CORES):
        loc, sr, cnt, pi, degs = percore[c]
        gidx = np.full(max(tot16, 16), zrow, np.int32)
        # rank of each dst in pi
        rank = np.empty(NG * 128, np.int64)
        rank[pi] = np.arange(NG * 128)
        # edges sorted by local dst for contiguous runs
        es = np.argsort(loc, kind="stable")
        loc_s, sr_s = loc[es], sr[es]
        starts = np.concatenate([[0], np.cumsum(cnt)])
        within = np.arange(loc_s.size) - starts[loc_s]
        r_dst = rank[loc_s]
        grp = r_dst // 128
        j = r_dst % 128
        slot = within
        pos = group_base[grp] + slot * 128 + j
        gidx[pos] = sr_s
        gidx_all.append(gidx[:max(tot16, 16)].astype(np.int16))
        # placement: token (batch, gi, p) -> dst pi[(batch*SC+gi)*128+p]
        pl = np.where(pi < sh, pi, shp - 1).astype(np.int16)  # pads -> trash row
        pidx_all.append(pl)
    return D_g.tolist(), gidx_all, pidx_all


def _wrap16(a):
    a = np.asarray(a)
    assert a.size % 16 == 0
    return np.tile(a.reshape(-1, 16).T, (8, 1)).copy()


def prepare(inputs, dims=DIMS):
    d = dims
    g_sh, d_sh, gp, dp = _shard_dims(d)
    MG, MD = gp // 128, dp // 128
    KTG, KTD = d["d_gene"] // 128, d["d_drug"] // 128
    KT1 = d["h0"] // 128
    h0, h1 = d["h0"], d["h1"]

    gene = np.asarray(inputs["gene_feat"], np.float32)
    drug = np.asarray(inputs["drug_feat"], np.float32)
    ei = {k: np.asarray(inputs[k], np.int64) for k in ("ei_gg", "ei_dd", "ei_dt", "ei_td")}

    dk = _tf_split(_tf_key(42), 8)
    inv_keep = np.float32(1.0 / (1.0 - P_DROP))

    def dropf(x, key):
        m = _tf_bernoulli(key, 1.0 - P_DROP, x.shape)
        return np.where(m, x * inv_keep, np.float32(0.0)).astype(np.float32)

    xg_gg = dropf(gene, dk[0])
    xd_dd = dropf(drug, dk[1])
    xd_dt = dropf(drug, dk[2])
    xg_td = dropf(gene, dk[3])
    # layer-1 masks (scaled): {0, 1.25}
    m1 = {
        "gg": _tf_bernoulli(dk[4], 1.0 - P_DROP, (d["n_gene"], h0)).astype(np.float32) * inv_keep,
        "dd": _tf_bernoulli(dk[5], 1.0 - P_DROP, (d["n_drug"], h0)).astype(np.float32) * inv_keep,
        "dt": _tf_bernoulli(dk[6], 1.0 - P_DROP, (d["n_drug"], h0)).astype(np.float32) * inv_keep,
        "td": _tf_bernoulli(dk[7], 1.0 - P_DROP, (d["n_gene"], h0)).astype(np.float32) * inv_keep,
    }

    # degrees (with self loop) for normalized types
    deg_g = np.bincount(ei["ei_gg"][1], minlength=d["n_gene"]).astype(np.float32) + 1.0
    deg_d = np.bincount(ei["ei_dd"][1], minlength=d["n_drug"]).astype(np.float32) + 1.0
    dinv_g = (1.0 / np.sqrt(deg_g)).astype(np.float32)
    dinv_d = (1.0 / np.sqrt(deg_d)).astype(np.float32)

    # gather-table row mapping (per-type tables, AllGather rank-major)
    def row_gene(s):
        return (s // g_sh) * gp + (s % g_sh)

    def row_dd(s):
        return (s // d_sh) * (2 * dp) + (s % d_sh)

    def row_dt(s):
        return (s // d_sh) * (2 * dp) + dp + (s % d_sh)

    ZG = g_sh          # rank-0 gg pad row (zero)
    ZD = d_sh          # rank-0 dd pad row (zero)

    st = {}
    st["gg"] = _edge_structure(ei["ei_gg"][0], ei["ei_gg"][1], d["n_gene"], g_sh, gp, row_gene, ZG)
    st["dd"] = _edge_structure(ei["ei_dd"][0], ei["ei_dd"][1], d["n_drug"], d_sh, dp, row_dd, ZD)
    st["dt"] = _edge_structure(ei["ei_dt"][0], ei["ei_dt"][1], d["n_gene"], g_sh, gp, row_dt, ZD)
    st["td"] = _edge_structure(ei["ei_td"][0], ei["ei_td"][1], d["n_drug"], d_sh, dp, row_gene, ZG)

    struct = dict(
        dims=d, g_sh=g_sh, d_sh=d_sh, gp=gp, dp=dp, MG=MG, MD=MD,
        KTG=KTG, KTD=KTD, KT1=KT1,
        Dg={et: st[et][0] for et in st},
    )

    def pack_lhsT(x, rows0, rows1, m_ch, kt):
        xc = np.zeros((m_ch * 128, kt * 128), np.float32)
        xc[: rows1 - rows0] = x[rows0:rows1]
        return np.ascontiguousarray(
            xc.reshape(m_ch, 128, kt, 128).transpose(0, 3, 2, 1)).astype(np.float16)

    def pack_w(w, kt, f):
        return np.ascontiguousarray(
            np.asarray(w, np.float32).reshape(kt, 128, f).transpose(1, 0, 2)).astype(np.float16)

    def pack_dinv(v, rows0, rows1, m_ch):
        z = np.zeros(m_ch * 128, np.float32)
        z[: rows1 - rows0] = v[rows0:rows1]
        return np.ascontiguousarray(z.reshape(m_ch, 128).T)

    def pad_rows(x, rows0, rows1, nrows):
        z = np.zeros((nrows, x.shape[1]), np.float32)
        z[: rows1 - rows0] = x[rows0:rows1]
        return z

    w16 = {f"w0_{et}": pack_w(inputs[f"W0_{et}"], (KTG if et in ("gg", "td") else KTD), h0)
           for et in ("gg", "dd", "dt", "td")}
    w16.update({f"w1_{et}": pack_w(inputs[f"W1_{et}"], KT1, h1) for et in ("gg", "dd", "dt", "td")})
    b32 = {f"b0_{et}": np.tile(np.asarray(inputs[f"b0_{et}"], np.float32), (128, 1))
           for et in ("gg", "dd", "dt", "td")}
    b32.update({f"b1_{et}": np.tile(np.asarray(inputs[f"b1_{et}"], np.float32), (128, 1))
                for et in ("gg", "dd", "dt", "td")})

    in_maps = []
    for c in range(N_CORES):
        gr0, gr1 = c * g_sh, (c + 1) * g_sh
        dr0, dr1 = c * d_sh, (c + 1) * d_sh
        im = dict(
            x_gg=pack_lhsT(xg_gg, gr0, gr1, MG, KTG),
            x_td=pack_lhsT(xg_td, gr0, gr1, MG, KTG),
            x_dd=pack_lhsT(xd_dd, dr0, dr1, MD, KTD),
            x_dt=pack_lhsT(xd_dt, dr0, dr1, MD, KTD),
            dinv_g=pack_dinv(dinv_g, gr0, gr1, MG),
            dinv_d=pack_dinv(dinv_d, dr0, dr1, MD),
            m1_gg=pad_rows(m1["gg"], gr0, gr1, gp),
            m1_td=pad_rows(m1["td"], gr0, gr1, gp),
            m1_dd=pad_rows(m1["dd"], dr0, dr1, dp),
            m1_dt=pad_rows(m1["dt"], dr0, dr1, dp),
            ident=np.eye(128, dtype=np.float32),
            **{k: v for k, v in w16.items()},
            **{k: v for k, v in b32.items()},
        )
        for et in ("gg", "dd", "dt", "td"):
            im[f"gidx_{et}"] = _wrap16(st[et][1][c])
            im[f"pidx_{et}"] = _wrap16(st[et][2][c])
        in_maps.append(im)
    return in_maps, struct


# ------------------------------------------------------------- kernel build ---

def build(struct, kreps=None, kskip=None):
    import concourse.bacc as bacc
    import concourse.mybir as mybir
    import concourse.tile as tile

    dt = mybir.dt
    ADD = mybir.AluOpType.add
    d = struct["dims"]
    gp, dp, MG, MD = struct["gp"], struct["dp"], struct["MG"], struct["MD"]
    KTG, KTD, KT1 = struct["KTG"], struct["KTD"], struct["KT1"]
    h0, h1 = d["h0"], d["h1"]
    Dg = struct["Dg"]

    nc = bacc.Bacc("TRN2", target_bir_lowering=False, debug=False, num_devices=N_CORES)

    def din(name, shape, dtype):
        return nc.dram_tensor(name, shape, dtype, kind="ExternalInput")

    x_gg = din("x_gg", [MG, 128, KTG, 128], dt.float16)
    x_td = din("x_td", [MG, 128, KTG, 128], dt.float16)
    x_dd = din("x_dd", [MD, 128, KTD, 128], dt.float16)
    x_dt = din("x_dt", [MD, 128, KTD, 128], dt.float16)
    dinv_g_d = din("dinv_g", [128, MG], dt.float32)
    dinv_d_d = din("dinv_d", [128, MD], dt.float32)
    m1_d = {et: din(f"m1_{et}", [gp if et in ("gg", "td") else dp, h0], dt.float32)
            for et in ("gg", "dd", "dt", "td")}
    ident_d = din("ident", [128, 128], dt.float32)
    w_d, b_d = {}, {}
    for et in ("gg", "dd", "dt", "td"):
        ktl0 = KTG if et in ("gg", "td") else KTD
        w_d[f"w0_{et}"] = din(f"w0_{et}", [128, ktl0, h0], dt.float16)
        w_d[f"w1_{et}"] = din(f"w1_{et}", [128, KT1, h1], dt.float16)
        b_d[f"b0_{et}"] = din(f"b0_{et}", [128, h0], dt.float32)
        b_d[f"b1_{et}"] = din(f"b1_{et}", [128, h1], dt.float32)
    gidx_d, pidx_d = {}, {}
    for et in ("gg", "dd", "dt", "td"):
        tot = max(int(sum(Dg[et]) * 128), 16)
        tot16 = ((tot + 15) // 16) * 16
        gidx_d[et] = din(f"gidx_{et}", [128, tot16 // 16], dt.int16)
        ng = (gp if et in ("gg", "dt") else dp) // 128
        pidx_d[et] = din(f"pidx_{et}", [128, ng * 8], dt.int16)

    gene_out = nc.dram_tensor("gene_out", [gp, h1], dt.float32, kind="ExternalOutput")
    drug_out = nc.dram_tensor("drug_out", [dp, h1], dt.float32, kind="ExternalOutput")

    # internal DRAM
    own, comb, acc, l1x = {}, {}, {}, {}
    for L, F in ((0, h0), (1, h1)):
        own[f"gg{L}"] = nc.dram_tensor(f"own_gg{L}", [gp, F], dt.float16)
        own[f"td{L}"] = nc.dram_tensor(f"own_td{L}", [gp, F], dt.float16)
        own[f"ddt{L}"] = nc.dram_tensor(f"own_ddt{L}", [2 * dp, F], dt.float16)
        comb[f"gg{L}"] = nc.dram_tensor(f"comb_gg{L}", [N_CORES * gp, F], dt.float16, addr_space="Shared")
        comb[f"td{L}"] = nc.dram_tensor(f"comb_td{L}", [N_CORES * gp, F], dt.float16, addr_space="Shared")
        comb[f"ddt{L}"] = nc.dram_tensor(f"comb_ddt{L}", [N_CORES * 2 * dp, F], dt.float16, addr_space="Shared")
        for et in ("gg", "dt"):
            acc[f"{et}{L}"] = nc.dram_tensor(f"acc_{et}{L}", [gp, F], dt.float32)
        for et in ("dd", "td"):
            acc[f"{et}{L}"] = nc.dram_tensor(f"acc_{et}{L}", [dp, F], dt.float32)
    for et in ("gg", "td"):
        l1x[et] = nc.dram_tensor(f"l1x_{et}", [MG, 128, KT1, 128], dt.float16)
    for et in ("dd", "dt"):
        l1x[et] = nc.dram_tensor(f"l1x_{et}", [MD, 128, KT1, 128], dt.float16)

    RELU = mybir.AluOpType.max

    with tile.TileContext(nc) as tc:
        with (
            tc.tile_pool(name="const", bufs=1) as cpool,
            tc.tile_pool(name="lhsT", bufs=3) as lpool,
            tc.tile_pool(name="epi", bufs=3) as epool,
            tc.tile_pool(name="gath", bufs=3) as gpool,
            tc.tile_pool(name="stage", bufs=2) as spool,
            tc.tile_pool(name="post", bufs=3) as qpool,
            tc.tile_pool(name="mm", bufs=4, space="PSUM") as mpool,
            tc.tile_pool(name="tp", bufs=4, space="PSUM") as tpool,
        ):
            # ---- resident constants ----
            ident = cpool.tile([128, 128], dt.float32, tag="ident")
            nc.sync.dma_start(ident[:], ident_d[:])
            dinv_g = cpool.tile([128, MG], dt.float32, tag="dinvg")
            nc.sync.dma_start(dinv_g[:], dinv_g_d[:])
            dinv_d = cpool.tile([128, MD], dt.float32, tag="dinvd")
            nc.sync.dma_start(dinv_d[:], dinv_d_d[:])
            wt, bt = {}, {}
            for k, tens in w_d.items():
                sh = tens.shape
                wt[k] = cpool.tile(list(sh), dt.float16, tag=f"w_{k}", name=f"wt_{k}")
                nc.sync.dma_start(wt[k][:], tens[:])
            for k, tens in b_d.items():
                bt[k] = cpool.tile(list(tens.shape), dt.float32, tag=f"b_{k}", name=f"bt_{k}")
                nc.sync.dma_start(bt[k][:], tens[:])
            gidx_t, pidx_t = {}, {}
            for et in ("gg", "dd", "dt", "td"):
                gidx_t[et] = cpool.tile(list(gidx_d[et].shape), dt.int16, tag=f"gi_{et}", name=f"gidx_t_{et}")
                nc.sync.dma_start(gidx_t[et][:], gidx_d[et][:])
                pidx_t[et] = cpool.tile(list(pidx_d[et].shape), dt.int16, tag=f"pi_{et}", name=f"pidx_t_{et}")
                nc.sync.dma_start(pidx_t[et][:], pidx_d[et][:])

            def l2norm(t, F):
                """In-place L2 row normalize of t [128, F] f32."""
                sq = qpool.tile([128, F], dt.float32, tag="l2sq")
                ssq = qpool.tile([128, 1], dt.float32, tag="l2ssq")
                nc.scalar.activation(sq[:], t[:], mybir.ActivationFunctionType.Square,
                                     accum_out=ssq[:])
                nrm = qpool.tile([128, 1], dt.float32, tag="l2n")
                nc.scalar.sqrt(nrm[:], ssq[:])
                nc.vector.tensor_scalar_max(nrm[:], nrm[:], 1e-12)
                rcp = qpool.tile([128, 1], dt.float32, tag="l2r")
                nc.vector.reciprocal(rcp[:], nrm[:])
                nc.vector.tensor_scalar_mul(t[:], t[:], rcp[:])

            def emit_reduce(gt, s, out_ap, accumulate):
                """Reduce gt[:, 0:s, :] (f16) along slots into out_ap [128,F] f32."""
                F = gt.shape[-1]
                while s > 2:
                    p = 1 << (int(s).bit_length() - 1)
                    if p == s:
                        p = s // 2
                    rem = s - p
                    nc.vector.tensor_tensor(
                        gt[:, 0:rem, :], gt[:, 0:rem, :], gt[:, p:p + rem, :], op=ADD)
                    s = p
                tmp = None
                tgt = out_ap
                if accumulate:
                    tmp = qpool.tile([128, F], dt.float32, tag="rtmp")
                    tgt = tmp[:]
                if s == 2:
                    nc.vector.tensor_tensor(
                        tgt,
                        gt[:, 0:1, :].rearrange("p a f -> p (a f)"),
                        gt[:, 1:2, :].rearrange("p a f -> p (a f)"), op=ADD)
                else:
                    nc.scalar.copy(tgt, gt[:, 0:1, :].rearrange("p a f -> p (a f)"))
                if accumulate:
                    nc.vector.tensor_add(out_ap, out_ap, tmp[:])

            def matmul_phase(L, F):
                KX = {"gg": KTG, "td": KTG, "dd": KTD, "dt": KTD} if L == 0 else \
                     {"gg": KT1, "td": KT1, "dd": KT1, "dt": KT1}
                XS = {"gg": x_gg, "td": x_td, "dd": x_dd, "dt": x_dt} if L == 0 else l1x
                for et in ("gg", "dd", "dt", "td"):
                    kt = KX[et]
                    m_ch = MG if et in ("gg", "td") else MD
                    w = wt[f"w{L}_{et}"]
                    for m in range(m_ch):
                        lt = lpool.tile([128, kt * 128], dt.float16, tag="lhsT")
                        nc.sync.dma_start(
                            lt[:], XS[et][m].rearrange("p t j -> p (t j)"))
                        ps = mpool.tile([128, F], dt.float32, tag="mmps")
                        for t in range(kt):
                            nc.tensor.matmul(ps[:], lt[:, t * 128:(t + 1) * 128],
                                             w[:, t, :], start=(t == 0), stop=(t == kt - 1))
                        r0, r1 = m * 128, (m + 1) * 128
                        if et in ("gg", "dd"):
                            dv = (dinv_g if et == "gg" else dinv_d)
                            s16 = epool.tile([128, F], dt.float16, tag="e16")
                            nc.vector.tensor_scalar_mul(s16[:], ps[:], dv[:, m:m + 1])
                            s32 = epool.tile([128, F], dt.float32, tag="e32")
                            nc.vector.tensor_scalar_mul(s32[:], ps[:], dv[:, m:m + 1])
                            tab = own[f"gg{L}"] if et == "gg" else own[f"ddt{L}"]
                            off = 0
                            nc.sync.dma_start(tab[off + r0:off + r1, :], s16[:])
                            nc.sync.dma_start(acc[f"{et}{L}"][r0:r1, :], s32[:])
                        else:
                            s16 = epool.tile([128, F], dt.float16, tag="e16")
                            nc.scalar.copy(s16[:], ps[:])
                            if et == "td":
                                nc.sync.dma_start(own[f"td{L}"][r0:r1, :], s16[:])
                            else:
                                nc.sync.dma_start(own[f"ddt{L}"][dp + r0:dp + r1, :], s16[:])
                # bias init for the un-normalized accumulators (dst-space!)
                for m in range(MG):
                    nc.sync.dma_start(acc[f"dt{L}"][m * 128:(m + 1) * 128, :], bt[f"b{L}_dt"][:])
                for m in range(MD):
                    nc.sync.dma_start(acc[f"td{L}"][m * 128:(m + 1) * 128, :], bt[f"b{L}_td"][:])

            def allgather_phase(L):
                for key in ("gg", "td", "ddt"):
                    nc.gpsimd.collective_compute(
                        "AllGather", mybir.AluOpType.bypass,
                        replica_groups=[list(range(N_CORES))],
                        ins=[own[f"{key}{L}"][:]],
                        outs=[comb[f"{key}{L}"][:]],
                    )

            def edge_phase(L, F):
                srctab = {"gg": comb[f"gg{L}"], "td": comb[f"td{L}"],
                          "dd": comb[f"ddt{L}"], "dt": comb[f"ddt{L}"]}
                for et in ("gg", "dd", "dt", "td"):
                    prof = Dg[et]
                    NGr = len(prof)
                    base_slot = 0
                    stage = None
                    sc0 = 0
                    for g in range(NGr):
                        if g % SC_GROUPS == 0:
                            stage = spool.tile([128, SC_GROUPS, F], dt.float32, tag=f"st")
                            sc0 = g
                        D = int(prof[g])
                        dest = stage[:, g - sc0, :]
                        if D == 0:
                            nc.gpsimd.memset(dest, 0.0)
                        nrounds = (D + ROUND_SLOTS - 1) // ROUND_SLOTS
                        for r in range(nrounds):
                            s0 = r * ROUND_SLOTS
                            ns = min(ROUND_SLOTS, D - s0)
                            gt = gpool.tile([128, ROUND_SLOTS, F], dt.float16, tag="gt")
                            c0 = 0
                            while c0 < ns:
                                cs = min(CALL_SLOTS, ns - c0)
                                slot0 = base_slot + s0 + c0
                                col0 = slot0 * 8  # 128 idx / 16
                                nc.gpsimd.dma_gather(
                                    gt[:, c0:c0 + cs, :], srctab[et][:],
                                    gidx_t[et][:, col0:col0 + cs * 8],
                                    num_idxs=cs * 128, num_idxs_reg=cs * 128,
                                    elem_size=F)
                                c0 += cs
                            emit_reduce(gt, ns, dest, accumulate=(r > 0))
                        base_slot += D
                        if g % SC_GROUPS == SC_GROUPS - 1 or g == NGr - 1:
                            ngr = g - sc0 + 1
                            nc.gpsimd.dma_scatter_add(
                                acc[f"{et}{L}"][:],
                                stage[:, 0:ngr, :],
                                pidx_t[et][:, sc0 * 8:(sc0 + ngr) * 8],
                                num_idxs=ngr * 128, num_idxs_reg=ngr * 128,
                                elem_size=F)

            def post_phase(L, F):
                # genes: gg + dt ; drugs: dd + td
                for kind, m_ch, a_key, b_key, dv, outt in (
                    ("g", MG, "gg", "dt", dinv_g, gene_out),
                    ("d", MD, "dd", "td", dinv_d, drug_out),
                ):
                    for m in range(m_ch):
                        r0, r1 = m * 128, (m + 1) * 128
                        ta = qpool.tile([128, F], dt.float32, tag="pa")
                        nc.sync.dma_start(ta[:], acc[f"{a_key}{L}"][r0:r1, :])
                        nc.vector.tensor_scalar_mul(ta[:], ta[:], dv[:, m:m + 1])
                        nc.vector.tensor_add(ta[:], ta[:], bt[f"b{L}_{a_key}"][:])
                        nc.vector.tensor_scalar_max(ta[:], ta[:], 0.0)
                        l2norm(ta, F)
                        tb = qpool.tile([128, F], dt.float32, tag="pb")
                        nc.sync.dma_start(tb[:], acc[f"{b_key}{L}"][r0:r1, :])
                        nc.vector.tensor_scalar_max(tb[:], tb[:], 0.0)
                        l2norm(tb, F)
                        nc.vector.tensor_add(ta[:], ta[:], tb[:])
                        if L == 0:
                            ets = ("gg", "td") if kind == "g" else ("dd", "dt")
                            for et in ets:
                                mk = qpool.tile([128, F], dt.float32, tag="mk")
                                nc.sync.dma_start(mk[:], m1_d[et][r0:r1, :])
                                xm = qpool.tile([128, F], dt.float32, tag="xm")
                                nc.vector.tensor_mul(xm[:], ta[:], mk[:])
                                for t in range(KT1):
                                    tp = tpool.tile([128, 128], dt.float32, tag="tp")
                                    nc.tensor.transpose(
                                        tp[:], xm[:, t * 128:(t + 1) * 128], ident[:])
                                    h16 = qpool.tile([128, 128], dt.float16, tag="h16")
                                    nc.scalar.copy(h16[:], tp[:])
                                    nc.sync.dma_start(l1x[et][m, :, t, :], h16[:])
                        else:
                            nc.sync.dma_start(outt[r0:r1, :], ta[:])

            import os as _os
            _skip = set(_os.environ.get("KSKIP", "").split(",")) if kskip is None else set(kskip)
            _reps = int(_os.environ.get("KREPS", "1")) if kreps is None else kreps
            for _rep in range(_reps):
                if "mm" not in _skip:
                    matmul_phase(0, h0)
                if "ag" not in _skip:
                    allgather_phase(0)
                if "edge" not in _skip:
                    edge_phase(0, h0)
                if "post" not in _skip:
                    post_phase(0, h0)
                if "mm" not in _skip:
                    matmul_phase(1, h1)
                if "ag" not in _skip:
                    allgather_phase(1)
                if "edge" not in _skip:
                    edge_phase(1, h1)
                if "post" not in _skip:
                    post_phase(1, h1)

    nc.compile()
    return nc


# -------------------------------------------------------------------- run ---

_CACHE = {}


def _make_runner(nc, in_maps):
    """Build a reusable sharded PJRT callable for nc (axon path)."""
    import jax
    import numpy as np
    from jax.sharding import Mesh, PartitionSpec, NamedSharding
    try:
        from jax.experimental.shard_map import shard_map
    except ImportError:
        from jax import shard_map
    from concourse import bass2jax
    import concourse.mybir as mybir

    bass2jax.install_neuronx_cc_hook()
    assert nc.dbg_addr is None

    partition_name = nc.partition_id_tensor.name if nc.partition_id_tensor else None
    in_names, out_names, out_avals, zero_outs = [], [], [], []
    for alloc in nc.m.functions[0].allocations:
        if not isinstance(alloc, mybir.MemoryLocationSet):
            continue
        name = alloc.memorylocations[0].name
        if alloc.kind == "ExternalInput":
            if name != partition_name:
                in_names.append(name)
        elif alloc.kind == "ExternalOutput":
            out_names.append(name)
            shape = tuple(alloc.tensor_shape)
            dtype = mybir.dt.np(alloc.dtype)
            out_avals.append(jax.core.ShapedArray(shape, dtype))
            zero_outs.append(np.zeros(shape, dtype))
    n_params = len(in_names)
    n_outs = len(out_avals)
    all_in_names = list(in_names) + list(out_names)
    if partition_name is not None:
        all_in_names.append(partition_name)

    def _body(*args):
        operands = list(args)
        if partition_name is not None:
            operands.append(bass2jax.partition_id_tensor())
        outs = bass2jax._bass_exec_p.bind(
            *operands,
            out_avals=tuple(out_avals),
            in_names=tuple(all_in_names),
            out_names=tuple(out_names),
            lowering_input_output_aliases=(),
            sim_require_finite=True,
            sim_require_nnan=True,
            nc=nc,
        )
        return tuple(outs)

    donate = tuple(range(n_params, n_params + n_outs))
    devices = jax.devices()[:N_CORES]
    mesh = Mesh(np.asarray(devices), ("core",))
    in_specs = (PartitionSpec("core"),) * (n_params + n_outs)
    out_specs = (PartitionSpec("core"),) * n_outs
    sharded = jax.jit(
        shard_map(_body, mesh=mesh, in_specs=in_specs, out_specs=out_specs,
                  check_rep=False),
        donate_argnums=donate, keep_unused=True)

    sh = NamedSharding(mesh, PartitionSpec("core"))
    concat_in = []

    def set_inputs(maps):
        concat_in.clear()
        concat_in.extend(
            jax.device_put(
                np.concatenate([np.asarray(maps[c][nm]) for c in range(N_CORES)], axis=0), sh)
            for nm in in_names)

    set_inputs(in_maps)

    def make_zeros():
        return [jax.device_put(np.zeros((N_CORES * z.shape[0], *z.shape[1:]), z.dtype), sh)
                for z in zero_outs]

    def run_once():
        outs = sharded(*concat_in, *make_zeros())
        return outs

    def to_results(outs):
        res = []
        arrs = [np.asarray(o) for o in outs]
        for c in range(N_CORES):
            res.append({nm: arrs[i].reshape(N_CORES, *out_avals[i].shape)[c]
                        for i, nm in enumerate(out_names)})
        return res

    return dict(sharded=sharded, concat_in=concat_in, make_zeros=make_zeros,
                run_once=run_once, to_results=to_results, set_inputs=set_inputs)


def _struct_key(struct):
    return repr(sorted(struct["Dg"].items())) + repr(sorted(struct["dims"].items()))


def _get_compiled(inputs, dims):
    in_maps, struct = prepare(inputs, dims)
    key = _struct_key(struct)
    if _CACHE.get("key") != key:
        nc = build(struct)
        _CACHE.update(key=key, nc=nc, struct=struct, runner=None)
    return _CACHE["nc"], _CACHE["struct"], in_maps


def _get_runner(nc, in_maps):
    if _CACHE.get("runner") is None:
        _CACHE["runner"] = _make_runner(nc, in_maps)
    else:
        _CACHE["runner"]["set_inputs"](in_maps)
    return _CACHE["runner"]


def run_timed(n=10):
    """Re-execute the compiled kernel n times; returns wall seconds per run."""
    import time
    r = _CACHE["runner"]
    zero_sets = [r["make_zeros"]() for _ in range(n)]
    for z in zero_sets:
        for a in z:
            a.block_until_ready()
    times = []
    for i in range(n):
        t0 = time.perf_counter()
        outs = r["sharded"](*r["concat_in"], *zero_sets[i])
        for o in outs:
            o.block_until_ready()
        times.append(time.perf_counter() - t0)
    return times


def _assemble(results, struct):
    d = struct["dims"]
    g_sh, d_sh = struct["g_sh"], struct["d_sh"]
    h1 = d["h1"]
    xg = np.concatenate([results[c]["gene_out"][:g_sh] for c in range(N_CORES)], axis=0)
    xd = np.concatenate([results[c]["drug_out"][:d_sh] for c in range(N_CORES)], axis=0)
    return np.concatenate([xg, xd], axis=0).astype(np.float32)


def kernel(**inputs):
    nc, struct, in_maps = _get_compiled(inputs, DIMS)
    from concourse.bass_utils import run_bass_kernel_spmd
    res = run_bass_kernel_spmd(nc, in_maps, list(range(N_CORES)))
    return _assemble(res.results, struct)


# revision 12
# speedup vs baseline: 2.2754x; 2.2754x over previous
"""Trainium2 Bass kernel for the 2-layer heterogeneous GCN encoder
(gene/drug graph). Self-contained: host-side prep (numpy), Bass/Tile
kernel build, SPMD execution on 8 NeuronCores.

Strategy:
 - Destination-row sharding: core c owns genes [2500c, 2500(c+1)) and
   drugs [375c, 375(c+1)) (padded to 2560/384 rows).
 - Per layer: each core computes its row-shard of X@W per edge type
   (fp16 matmul, fp32 psum), scaled by dinv for the normalized types;
   AllGather the per-type message tables (fp16); per-core gather of
   source rows (SWDGE dma_gather), on-chip segment reduction (DVE add
   tree along the free dim, one dst per partition), placement into
   fp32 accumulators via unique-index dma_scatter_add; dense post
   (dinv scale, bias, relu, l2norm, type-sum).
 - GCN norm factorized: D^-1/2(A+I)D^-1/2 XW = dinv*(A_scaled + self),
   so no per-edge multiply is needed.
 - Dropout masks are deterministic (threefry, key 42) and computed on
   host in pure numpy (bit-exact vs jax), pre-applied to layer-0
   inputs and to layer-1 activations before the layer-1 matmul.
"""
import numpy as np

N_CORES = 8

# ---------------------------------------------------------------- threefry ---

def _rotl(x, r):
    return ((x << np.uint32(r)) | (x >> np.uint32(32 - r))).astype(np.uint32)

_ROT = [[13, 15, 26, 6], [17, 29, 16, 24]]

def _threefry_core(keypair, x0, x1):
    x0 = x0.astype(np.uint32).copy()
    x1 = x1.astype(np.uint32).copy()
    ks0, ks1 = np.uint32(keypair[0]), np.uint32(keypair[1])
    ks2 = np.uint32(ks0 ^ ks1 ^ np.uint32(0x1BD11BDA))
    x0 = (x0 + ks0).astype(np.uint32)
    x1 = (x1 + ks1).astype(np.uint32)
    ks = [ks1, ks2, ks0]
    for i in range(5):
        for r in _ROT[i % 2]:
            x0 = (x0 + x1).astype(np.uint32)
            x1 = _rotl(x1, r)
            x1 = (x1 ^ x0).astype(np.uint32)
        x0 = (x0 + ks[i % 3]).astype(np.uint32)
        x1 = (x1 + ks[(i + 1) % 3] + np.uint32(i + 1)).astype(np.uint32)
    return x0, x1

def _tf_key(seed):
    return (np.uint32(seed >> 32), np.uint32(seed & 0xFFFFFFFF))

def _tf_split(k, num):
    hi, lo = _threefry_core(k, np.zeros(num, np.uint32), np.arange(num, dtype=np.uint32))
    return [(hi[i], lo[i]) for i in range(num)]

def _tf_bernoulli(k, p, shape):
    n = int(np.prod(shape))
    assert n < 2**32
    hi, lo = _threefry_core(k, np.zeros(n, np.uint32), np.arange(n, dtype=np.uint32))
    bits = (hi ^ lo)
    fl = ((bits >> np.uint32(9)) | np.uint32(0x3F800000)).view(np.float32) - np.float32(1.0)
    return (fl < np.float32(p)).reshape(shape)

# ------------------------------------------------------------------- config ---

DIMS = dict(n_gene=20000, n_drug=3000, d_gene=1024, d_drug=512, h0=256, h1=128)
P_DROP = 0.2
CALL_SLOTS = 8      # 8*128 = 1024 gather idxs per SWDGE call (desc-ring cap)
ROUND_SLOTS = 32    # slots per reduce round (gather tile free size)
SC_GROUPS = 4       # groups per placement scatter (4*128 = 512 tokens)


def _pad128(n):
    return ((n + 127) // 128) * 128


def _shard_dims(dims):
    g_sh = dims["n_gene"] // N_CORES
    d_sh = dims["n_drug"] // N_CORES
    gp, dp = _pad128(g_sh), _pad128(d_sh)
    if gp == g_sh:
        gp += 128  # need trash rows
    if dp == d_sh:
        dp += 128
    return g_sh, d_sh, gp, dp


# ---------------------------------------------------------------- host prep ---

def _edge_structure(src, dst, n_dst_total, sh, shp, table_row_of_src, zrow):
    """Per-core gather/placement structure for one edge type.

    Returns (D_g_profile (len NG), gidx_percore [8][Ntot] int16,
    pidx_percore [8][NG*128] int16)."""
    NG = shp // 128
    owner = dst // sh
    local = dst % sh
    srows = table_row_of_src(src)

    # per-core per-dst counts and sorted order
    percore = []
    for c in range(N_CORES):
        sel = owner == c
        loc = local[sel]
        sr = srows[sel]
        cnt = np.bincount(loc, minlength=sh)
        order = np.argsort(-cnt, kind="stable")  # real dsts, degree desc
        pi = np.concatenate([order, np.arange(sh, NG * 128)])  # pad dsts at end
        degs = np.concatenate([cnt[order], np.zeros(NG * 128 - sh, np.int64)])
        percore.append((loc, sr, cnt, pi, degs))

    D_g = np.zeros(NG, np.int64)
    for c in range(N_CORES):
        degs = percore[c][4]
        for g in range(NG):
            D_g[g] = max(D_g[g], degs[g * 128:(g + 1) * 128].max())
    # total idx slots: sum over groups of D_g * 128
    tot = int(D_g.sum() * 128)
    tot16 = ((tot + 15) // 16) * 16

    group_base = np.concatenate([[0], np.cumsum(D_g * 128)]).astype(np.int64)

    gidx_all, pidx_all = [], []
    for c in range(N_CORES):
        loc, sr, cnt, pi, degs = percore[c]
        gidx = np.full(max(tot16, 16), zrow, np.int32)
        # rank of each dst in pi
        rank = np.empty(NG * 128, np.int64)
        rank[pi] = np.arange(NG * 128)
        # edges sorted by local dst for contiguous runs
        es = np.argsort(loc, kind="stable")
        loc_s, sr_s = loc[es], sr[es]
        starts = np.concatenate([[0], np.cumsum(cnt)])
        within = np.arange(loc_s.size) - starts[loc_s]
        r_dst = rank[loc_s]
        grp = r_dst // 128
        j = r_dst % 128
        slot = within
        pos = group_base[grp] + slot * 128 + j
        gidx[pos] = sr_s
        gidx_all.append(gidx[:max(tot16, 16)].astype(np.int16))
        # placement: token (batch, gi, p) -> dst pi[(batch*SC+gi)*128+p]
        pl = np.where(pi < sh, pi, shp - 1).astype(np.int16)  # pads -> trash row
        pidx_all.append(pl)
    return D_g.tolist(), gidx_all, pidx_all


def _wrap16(a):
    a = np.asarray(a)
    assert a.size % 16 == 0
    return np.tile(a.reshape(-1, 16).T, (8, 1)).copy()


def prepare(inputs, dims=DIMS):
    d = dims
    g_sh, d_sh, gp, dp = _shard_dims(d)
    MG, MD = gp // 128, dp // 128
    KTG, KTD = d["d_gene"] // 128, d["d_drug"] // 128
    KT1 = d["h0"] // 128
    h0, h1 = d["h0"], d["h1"]

    gene = np.asarray(inputs["gene_feat"], np.float32)
    drug = np.asarray(inputs["drug_feat"], np.float32)
    ei = {k: np.asarray(inputs[k], np.int64) for k in ("ei_gg", "ei_dd", "ei_dt", "ei_td")}

    dk = _tf_split(_tf_key(42), 8)
    inv_keep = np.float32(1.0 / (1.0 - P_DROP))

    def dropf(x, key):
        m = _tf_bernoulli(key, 1.0 - P_DROP, x.shape)
        return np.where(m, x * inv_keep, np.float32(0.0)).astype(np.float32)

    xg_gg = dropf(gene, dk[0])
    xd_dd = dropf(drug, dk[1])
    xd_dt = dropf(drug, dk[2])
    xg_td = dropf(gene, dk[3])
    # layer-1 masks (scaled): {0, 1.25}
    m1 = {
        "gg": _tf_bernoulli(dk[4], 1.0 - P_DROP, (d["n_gene"], h0)).astype(np.float32) * inv_keep,
        "dd": _tf_bernoulli(dk[5], 1.0 - P_DROP, (d["n_drug"], h0)).astype(np.float32) * inv_keep,
        "dt": _tf_bernoulli(dk[6], 1.0 - P_DROP, (d["n_drug"], h0)).astype(np.float32) * inv_keep,
        "td": _tf_bernoulli(dk[7], 1.0 - P_DROP, (d["n_gene"], h0)).astype(np.float32) * inv_keep,
    }

    # degrees (with self loop) for normalized types
    deg_g = np.bincount(ei["ei_gg"][1], minlength=d["n_gene"]).astype(np.float32) + 1.0
    deg_d = np.bincount(ei["ei_dd"][1], minlength=d["n_drug"]).astype(np.float32) + 1.0
    dinv_g = (1.0 / np.sqrt(deg_g)).astype(np.float32)
    dinv_d = (1.0 / np.sqrt(deg_d)).astype(np.float32)

    # gather-table row mapping (per-type tables, AllGather rank-major)
    def row_gene(s):
        return (s // g_sh) * gp + (s % g_sh)

    def row_dd(s):
        return (s // d_sh) * (2 * dp) + (s % d_sh)

    def row_dt(s):
        return (s // d_sh) * (2 * dp) + dp + (s % d_sh)

    ZG = g_sh          # rank-0 gg pad row (zero)
    ZD = d_sh          # rank-0 dd pad row (zero)

    st = {}
    st["gg"] = _edge_structure(ei["ei_gg"][0], ei["ei_gg"][1], d["n_gene"], g_sh, gp, row_gene, ZG)
    st["dd"] = _edge_structure(ei["ei_dd"][0], ei["ei_dd"][1], d["n_drug"], d_sh, dp, row_dd, ZD)
    st["dt"] = _edge_structure(ei["ei_dt"][0], ei["ei_dt"][1], d["n_gene"], g_sh, gp, row_dt, ZD)
    st["td"] = _edge_structure(ei["ei_td"][0], ei["ei_td"][1], d["n_drug"], d_sh, dp, row_gene, ZG)

    struct = dict(
        dims=d, g_sh=g_sh, d_sh=d_sh, gp=gp, dp=dp, MG=MG, MD=MD,
        KTG=KTG, KTD=KTD, KT1=KT1,
        Dg={et: st[et][0] for et in st},
    )

    def pack_lhsT(x, rows0, rows1, m_ch, kt):
        xc = np.zeros((m_ch * 128, kt * 128), np.float32)
        xc[: rows1 - rows0] = x[rows0:rows1]
        return np.ascontiguousarray(
            xc.reshape(m_ch, 128, kt, 128).transpose(0, 3, 2, 1)).astype(np.float16)

    def pack_w(w, kt, f):
        return np.ascontiguousarray(
            np.asarray(w, np.float32).reshape(kt, 128, f).transpose(1, 0, 2)).astype(np.float16)

    def pack_dinv(v, rows0, rows1, m_ch):
        z = np.zeros(m_ch * 128, np.float32)
        z[: rows1 - rows0] = v[rows0:rows1]
        return np.ascontiguousarray(z.reshape(m_ch, 128).T)

    def pad_rows(x, rows0, rows1, nrows):
        z = np.zeros((nrows, x.shape[1]), np.float32)
        z[: rows1 - rows0] = x[rows0:rows1]
        return z

    w16 = {f"w0_{et}": pack_w(inputs[f"W0_{et}"], (KTG if et in ("gg", "td") else KTD), h0)
           for et in ("gg", "dd", "dt", "td")}
    w16.update({f"w1_{et}": pack_w(inputs[f"W1_{et}"], KT1, h1) for et in ("gg", "dd", "dt", "td")})
    b32 = {f"b0_{et}": np.tile(np.asarray(inputs[f"b0_{et}"], np.float32), (128, 1))
           for et in ("gg", "dd", "dt", "td")}
    b32.update({f"b1_{et}": np.tile(np.asarray(inputs[f"b1_{et}"], np.float32), (128, 1))
                for et in ("gg", "dd", "dt", "td")})

    in_maps = []
    for c in range(N_CORES):
        gr0, gr1 = c * g_sh, (c + 1) * g_sh
        dr0, dr1 = c * d_sh, (c + 1) * d_sh
        im = dict(
            x_gg=pack_lhsT(xg_gg, gr0, gr1, MG, KTG),
            x_td=pack_lhsT(xg_td, gr0, gr1, MG, KTG),
            x_dd=pack_lhsT(xd_dd, dr0, dr1, MD, KTD),
            x_dt=pack_lhsT(xd_dt, dr0, dr1, MD, KTD),
            dinv_g=pack_dinv(dinv_g, gr0, gr1, MG),
            dinv_d=pack_dinv(dinv_d, dr0, dr1, MD),
            m1_gg=pad_rows(m1["gg"], gr0, gr1, gp),
            m1_td=pad_rows(m1["td"], gr0, gr1, gp),
            m1_dd=pad_rows(m1["dd"], dr0, dr1, dp),
            m1_dt=pad_rows(m1["dt"], dr0, dr1, dp),
            ident=np.eye(128, dtype=np.float32),
            **{k: v for k, v in w16.items()},
            **{k: v for k, v in b32.items()},
        )
        for et in ("gg", "dd", "dt", "td"):
            im[f"gidx_{et}"] = _wrap16(st[et][1][c])
            im[f"pidx_{et}"] = _wrap16(st[et][2][c])
        in_maps.append(im)
    return in_maps, struct


# ------------------------------------------------------------- kernel build ---

def build(struct, kreps=None, kskip=None):
    import concourse.bacc as bacc
    import concourse.mybir as mybir
    import concourse.tile as tile

    dt = mybir.dt
    ADD = mybir.AluOpType.add
    d = struct["dims"]
    gp, dp, MG, MD = struct["gp"], struct["dp"], struct["MG"], struct["MD"]
    KTG, KTD, KT1 = struct["KTG"], struct["KTD"], struct["KT1"]
    h0, h1 = d["h0"], d["h1"]
    Dg = struct["Dg"]

    nc = bacc.Bacc("TRN2", target_bir_lowering=False, debug=False, num_devices=N_CORES)

    def din(name, shape, dtype):
        return nc.dram_tensor(name, shape, dtype, kind="ExternalInput")

    x_gg = din("x_gg", [MG, 128, KTG, 128], dt.float16)
    x_td = din("x_td", [MG, 128, KTG, 128], dt.float16)
    x_dd = din("x_dd", [MD, 128, KTD, 128], dt.float16)
    x_dt = din("x_dt", [MD, 128, KTD, 128], dt.float16)
    dinv_g_d = din("dinv_g", [128, MG], dt.float32)
    dinv_d_d = din("dinv_d", [128, MD], dt.float32)
    m1_d = {et: din(f"m1_{et}", [gp if et in ("gg", "td") else dp, h0], dt.float32)
            for et in ("gg", "dd", "dt", "td")}
    ident_d = din("ident", [128, 128], dt.float32)
    w_d, b_d = {}, {}
    for et in ("gg", "dd", "dt", "td"):
        ktl0 = KTG if et in ("gg", "td") else KTD
        w_d[f"w0_{et}"] = din(f"w0_{et}", [128, ktl0, h0], dt.float16)
        w_d[f"w1_{et}"] = din(f"w1_{et}", [128, KT1, h1], dt.float16)
        b_d[f"b0_{et}"] = din(f"b0_{et}", [128, h0], dt.float32)
        b_d[f"b1_{et}"] = din(f"b1_{et}", [128, h1], dt.float32)
    gidx_d, pidx_d = {}, {}
    for et in ("gg", "dd", "dt", "td"):
        tot = max(int(sum(Dg[et]) * 128), 16)
        tot16 = ((tot + 15) // 16) * 16
        gidx_d[et] = din(f"gidx_{et}", [128, tot16 // 16], dt.int16)
        ng = (gp if et in ("gg", "dt") else dp) // 128
        pidx_d[et] = din(f"pidx_{et}", [128, ng * 8], dt.int16)

    gene_out = nc.dram_tensor("gene_out", [gp, h1], dt.float32, kind="ExternalOutput")
    drug_out = nc.dram_tensor("drug_out", [dp, h1], dt.float32, kind="ExternalOutput")

    # internal DRAM
    own, comb, acc, l1x = {}, {}, {}, {}
    for L, F in ((0, h0), (1, h1)):
        own[f"gg{L}"] = nc.dram_tensor(f"own_gg{L}", [gp, F], dt.float16)
        own[f"td{L}"] = nc.dram_tensor(f"own_td{L}", [gp, F], dt.float16)
        own[f"ddt{L}"] = nc.dram_tensor(f"own_ddt{L}", [2 * dp, F], dt.float16)
        comb[f"gg{L}"] = nc.dram_tensor(f"comb_gg{L}", [N_CORES * gp, F], dt.float16, addr_space="Shared")
        comb[f"td{L}"] = nc.dram_tensor(f"comb_td{L}", [N_CORES * gp, F], dt.float16, addr_space="Shared")
        comb[f"ddt{L}"] = nc.dram_tensor(f"comb_ddt{L}", [N_CORES * 2 * dp, F], dt.float16, addr_space="Shared")
        for et in ("gg", "dt"):
            acc[f"{et}{L}"] = nc.dram_tensor(f"acc_{et}{L}", [gp, F], dt.float32)
        for et in ("dd", "td"):
            acc[f"{et}{L}"] = nc.dram_tensor(f"acc_{et}{L}", [dp, F], dt.float32)
    for et in ("gg", "td"):
        l1x[et] = nc.dram_tensor(f"l1x_{et}", [MG, 128, KT1, 128], dt.float16)
    for et in ("dd", "dt"):
        l1x[et] = nc.dram_tensor(f"l1x_{et}", [MD, 128, KT1, 128], dt.float16)

    RELU = mybir.AluOpType.max

    with tile.TileContext(nc) as tc:
        with (
            tc.tile_pool(name="const", bufs=1) as cpool,
            tc.tile_pool(name="lhsT", bufs=4) as lpool,
            tc.tile_pool(name="epi", bufs=3) as epool,
            tc.tile_pool(name="gath", bufs=5) as gpool,
            tc.tile_pool(name="stage", bufs=3) as spool,
            tc.tile_pool(name="post", bufs=4) as qpool,
            tc.tile_pool(name="mm", bufs=4, space="PSUM") as mpool,
            tc.tile_pool(name="tp", bufs=4, space="PSUM") as tpool,
        ):
            # ---- resident constants ----
            ident = cpool.tile([128, 128], dt.float32, tag="ident")
            nc.sync.dma_start(ident[:], ident_d[:])
            dinv_g = cpool.tile([128, MG], dt.float32, tag="dinvg")
            nc.sync.dma_start(dinv_g[:], dinv_g_d[:])
            dinv_d = cpool.tile([128, MD], dt.float32, tag="dinvd")
            nc.sync.dma_start(dinv_d[:], dinv_d_d[:])
            wt, bt = {}, {}
            for k, tens in w_d.items():
                sh = tens.shape
                wt[k] = cpool.tile(list(sh), dt.float16, tag=f"w_{k}", name=f"wt_{k}")
                nc.sync.dma_start(wt[k][:], tens[:])
            for k, tens in b_d.items():
                bt[k] = cpool.tile(list(tens.shape), dt.float32, tag=f"b_{k}", name=f"bt_{k}")
                nc.sync.dma_start(bt[k][:], tens[:])
            gidx_t, pidx_t = {}, {}
            for et in ("gg", "dd", "dt", "td"):
                gidx_t[et] = cpool.tile(list(gidx_d[et].shape), dt.int16, tag=f"gi_{et}", name=f"gidx_t_{et}")
                nc.sync.dma_start(gidx_t[et][:], gidx_d[et][:])
                pidx_t[et] = cpool.tile(list(pidx_d[et].shape), dt.int16, tag=f"pi_{et}", name=f"pidx_t_{et}")
                nc.sync.dma_start(pidx_t[et][:], pidx_d[et][:])

            def l2norm(t, F):
                """In-place L2 row normalize of t [128, F] f32."""
                sq = qpool.tile([128, F], dt.float32, tag="l2sq")
                ssq = qpool.tile([128, 1], dt.float32, tag="l2ssq")
                nc.scalar.activation(sq[:], t[:], mybir.ActivationFunctionType.Square,
                                     accum_out=ssq[:])
                nrm = qpool.tile([128, 1], dt.float32, tag="l2n")
                nc.scalar.sqrt(nrm[:], ssq[:])
                nc.vector.tensor_scalar_max(nrm[:], nrm[:], 1e-12)
                rcp = qpool.tile([128, 1], dt.float32, tag="l2r")
                nc.vector.reciprocal(rcp[:], nrm[:])
                nc.vector.tensor_scalar_mul(t[:], t[:], rcp[:])

            def emit_reduce(gt, s, out_ap, accumulate):
                """Reduce gt[:, 0:s, :] (f16) along slots into out_ap [128,F] f32."""
                F = gt.shape[-1]
                while s > 2:
                    p = 1 << (int(s).bit_length() - 1)
                    if p == s:
                        p = s // 2
                    rem = s - p
                    nc.vector.tensor_tensor(
                        gt[:, 0:rem, :], gt[:, 0:rem, :], gt[:, p:p + rem, :], op=ADD)
                    s = p
                tmp = None
                tgt = out_ap
                if accumulate:
                    tmp = qpool.tile([128, F], dt.float32, tag="rtmp")
                    tgt = tmp[:]
                if s == 2:
                    nc.vector.tensor_tensor(
                        tgt,
                        gt[:, 0:1, :].rearrange("p a f -> p (a f)"),
                        gt[:, 1:2, :].rearrange("p a f -> p (a f)"), op=ADD)
                else:
                    nc.scalar.copy(tgt, gt[:, 0:1, :].rearrange("p a f -> p (a f)"))
                if accumulate:
                    nc.vector.tensor_add(out_ap, out_ap, tmp[:])

            def matmul_phase(L, F):
                KX = {"gg": KTG, "td": KTG, "dd": KTD, "dt": KTD} if L == 0 else \
                     {"gg": KT1, "td": KT1, "dd": KT1, "dt": KT1}
                XS = {"gg": x_gg, "td": x_td, "dd": x_dd, "dt": x_dt} if L == 0 else l1x
                for et in ("gg", "dd", "dt", "td"):
                    kt = KX[et]
                    m_ch = MG if et in ("gg", "td") else MD
                    w = wt[f"w{L}_{et}"]
                    for m in range(m_ch):
                        lt = lpool.tile([128, kt * 128], dt.float16, tag="lhsT")
                        nc.sync.dma_start(
                            lt[:], XS[et][m].rearrange("p t j -> p (t j)"))
                        ps = mpool.tile([128, F], dt.float32, tag="mmps")
                        for t in range(kt):
                            nc.tensor.matmul(ps[:], lt[:, t * 128:(t + 1) * 128],
                                             w[:, t, :], start=(t == 0), stop=(t == kt - 1))
                        r0, r1 = m * 128, (m + 1) * 128
                        if et in ("gg", "dd"):
                            dv = (dinv_g if et == "gg" else dinv_d)
                            s16 = epool.tile([128, F], dt.float16, tag="e16")
                            nc.vector.tensor_scalar_mul(s16[:], ps[:], dv[:, m:m + 1])
                            s32 = epool.tile([128, F], dt.float32, tag="e32")
                            nc.vector.tensor_scalar_mul(s32[:], ps[:], dv[:, m:m + 1])
                            tab = own[f"gg{L}"] if et == "gg" else own[f"ddt{L}"]
                            off = 0
                            nc.sync.dma_start(tab[off + r0:off + r1, :], s16[:])
                            nc.sync.dma_start(acc[f"{et}{L}"][r0:r1, :], s32[:])
                        else:
                            s16 = epool.tile([128, F], dt.float16, tag="e16")
                            nc.scalar.copy(s16[:], ps[:])
                            if et == "td":
                                nc.sync.dma_start(own[f"td{L}"][r0:r1, :], s16[:])
                            else:
                                nc.sync.dma_start(own[f"ddt{L}"][dp + r0:dp + r1, :], s16[:])
                # bias init for the un-normalized accumulators (dst-space!)
                for m in range(MG):
                    nc.sync.dma_start(acc[f"dt{L}"][m * 128:(m + 1) * 128, :], bt[f"b{L}_dt"][:])
                for m in range(MD):
                    nc.sync.dma_start(acc[f"td{L}"][m * 128:(m + 1) * 128, :], bt[f"b{L}_td"][:])

            def allgather_phase(L):
                for key in ("gg", "td", "ddt"):
                    nc.gpsimd.collective_compute(
                        "AllGather", mybir.AluOpType.bypass,
                        replica_groups=[list(range(N_CORES))],
                        ins=[own[f"{key}{L}"][:]],
                        outs=[comb[f"{key}{L}"][:]],
                    )

            def edge_phase(L, F):
                srctab = {"gg": comb[f"gg{L}"], "td": comb[f"td{L}"],
                          "dd": comb[f"ddt{L}"], "dt": comb[f"ddt{L}"]}
                for et in ("gg", "dd", "dt", "td"):
                    prof = Dg[et]
                    NGr = len(prof)
                    base_slot = 0
                    stage = None
                    sc0 = 0
                    for g in range(NGr):
                        if g % SC_GROUPS == 0:
                            stage = spool.tile([128, SC_GROUPS, F], dt.float32, tag=f"st")
                            sc0 = g
                        D = int(prof[g])
                        dest = stage[:, g - sc0, :]
                        if D == 0:
                            nc.gpsimd.memset(dest, 0.0)
                        nrounds = (D + ROUND_SLOTS - 1) // ROUND_SLOTS
                        for r in range(nrounds):
                            s0 = r * ROUND_SLOTS
                            ns = min(ROUND_SLOTS, D - s0)
                            gt = gpool.tile([128, ROUND_SLOTS, F], dt.float16, tag="gt")
                            c0 = 0
                            while c0 < ns:
                                cs = min(CALL_SLOTS, ns - c0)
                                slot0 = base_slot + s0 + c0
                                col0 = slot0 * 8  # 128 idx / 16
                                nc.gpsimd.dma_gather(
                                    gt[:, c0:c0 + cs, :], srctab[et][:],
                                    gidx_t[et][:, col0:col0 + cs * 8],
                                    num_idxs=cs * 128, num_idxs_reg=cs * 128,
                                    elem_size=F)
                                c0 += cs
                            emit_reduce(gt, ns, dest, accumulate=(r > 0))
                        base_slot += D
                        if g % SC_GROUPS == SC_GROUPS - 1 or g == NGr - 1:
                            ngr = g - sc0 + 1
                            nc.gpsimd.dma_scatter_add(
                                acc[f"{et}{L}"][:],
                                stage[:, 0:ngr, :],
                                pidx_t[et][:, sc0 * 8:(sc0 + ngr) * 8],
                                num_idxs=ngr * 128, num_idxs_reg=ngr * 128,
                                elem_size=F)

            def post_phase(L, F):
                # genes: gg + dt ; drugs: dd + td
                for kind, m_ch, a_key, b_key, dv, outt in (
                    ("g", MG, "gg", "dt", dinv_g, gene_out),
                    ("d", MD, "dd", "td", dinv_d, drug_out),
                ):
                    for m in range(m_ch):
                        r0, r1 = m * 128, (m + 1) * 128
                        ta = qpool.tile([128, F], dt.float32, tag="pa")
                        nc.sync.dma_start(ta[:], acc[f"{a_key}{L}"][r0:r1, :])
                        nc.vector.tensor_scalar_mul(ta[:], ta[:], dv[:, m:m + 1])
                        nc.vector.tensor_add(ta[:], ta[:], bt[f"b{L}_{a_key}"][:])
                        nc.vector.tensor_scalar_max(ta[:], ta[:], 0.0)
                        l2norm(ta, F)
                        tb = qpool.tile([128, F], dt.float32, tag="pb")
                        nc.sync.dma_start(tb[:], acc[f"{b_key}{L}"][r0:r1, :])
                        nc.vector.tensor_scalar_max(tb[:], tb[:], 0.0)
                        l2norm(tb, F)
                        nc.vector.tensor_add(ta[:], ta[:], tb[:])
                        if L == 0:
                            ets = ("gg", "td") if kind == "g" else ("dd", "dt")
                            for et in ets:
                                mk = qpool.tile([128, F], dt.float32, tag="mk")
                                nc.sync.dma_start(mk[:], m1_d[et][r0:r1, :])
                                xm = qpool.tile([128, F], dt.float32, tag="xm")
                                nc.vector.tensor_mul(xm[:], ta[:], mk[:])
                                for t in range(KT1):
                                    tp = tpool.tile([128, 128], dt.float32, tag="tp")
                                    nc.tensor.transpose(
                                        tp[:], xm[:, t * 128:(t + 1) * 128], ident[:])
                                    h16 = qpool.tile([128, 128], dt.float16, tag="h16")
                                    nc.scalar.copy(h16[:], tp[:])
                                    nc.sync.dma_start(l1x[et][m, :, t, :], h16[:])
                        else:
                            nc.sync.dma_start(outt[r0:r1, :], ta[:])

            import os as _os
            _skip = set(_os.environ.get("KSKIP", "").split(",")) if kskip is None else set(kskip)
            _reps = int(_os.environ.get("KREPS", "1")) if kreps is None else kreps
            for _rep in range(_reps):
                if "mm" not in _skip:
                    matmul_phase(0, h0)
                if "ag" not in _skip:
                    allgather_phase(0)
                if "edge" not in _skip:
                    edge_phase(0, h0)
                if "post" not in _skip:
                    post_phase(0, h0)
                if "mm" not in _skip:
                    matmul_phase(1, h1)
                if "ag" not in _skip:
                    allgather_phase(1)
                if "edge" not in _skip:
                    edge_phase(1, h1)
                if "post" not in _skip:
                    post_phase(1, h1)

    nc.compile()
    return nc


# -------------------------------------------------------------------- run ---

_CACHE = {}


def _make_runner(nc, in_maps):
    """Build a reusable sharded PJRT callable for nc (axon path)."""
    import jax
    import numpy as np
    from jax.sharding import Mesh, PartitionSpec, NamedSharding
    try:
        from jax.experimental.shard_map import shard_map
    except ImportError:
        from jax import shard_map
    from concourse import bass2jax
    import concourse.mybir as mybir

    bass2jax.install_neuronx_cc_hook()
    assert nc.dbg_addr is None

    partition_name = nc.partition_id_tensor.name if nc.partition_id_tensor else None
    in_names, out_names, out_avals, zero_outs = [], [], [], []
    for alloc in nc.m.functions[0].allocations:
        if not isinstance(alloc, mybir.MemoryLocationSet):
            continue
        name = alloc.memorylocations[0].name
        if alloc.kind == "ExternalInput":
            if name != partition_name:
                in_names.append(name)
        elif alloc.kind == "ExternalOutput":
            out_names.append(name)
            shape = tuple(alloc.tensor_shape)
            dtype = mybir.dt.np(alloc.dtype)
            out_avals.append(jax.core.ShapedArray(shape, dtype))
            zero_outs.append(np.zeros(shape, dtype))
    n_params = len(in_names)
    n_outs = len(out_avals)
    all_in_names = list(in_names) + list(out_names)
    if partition_name is not None:
        all_in_names.append(partition_name)

    def _body(*args):
        operands = list(args)
        if partition_name is not None:
            operands.append(bass2jax.partition_id_tensor())
        outs = bass2jax._bass_exec_p.bind(
            *operands,
            out_avals=tuple(out_avals),
            in_names=tuple(all_in_names),
            out_names=tuple(out_names),
            lowering_input_output_aliases=(),
            sim_require_finite=True,
            sim_require_nnan=True,
            nc=nc,
        )
        return tuple(outs)

    donate = tuple(range(n_params, n_params + n_outs))
    devices = jax.devices()[:N_CORES]
    mesh = Mesh(np.asarray(devices), ("core",))
    in_specs = (PartitionSpec("core"),) * (n_params + n_outs)
    out_specs = (PartitionSpec("core"),) * n_outs
    sharded = jax.jit(
        shard_map(_body, mesh=mesh, in_specs=in_specs, out_specs=out_specs,
                  check_rep=False),
        donate_argnums=donate, keep_unused=True)

    sh = NamedSharding(mesh, PartitionSpec("core"))
    concat_in = []

    def set_inputs(maps):
        concat_in.clear()
        concat_in.extend(
            jax.device_put(
                np.concatenate([np.asarray(maps[c][nm]) for c in range(N_CORES)], axis=0), sh)
            for nm in in_names)

    set_inputs(in_maps)

    def make_zeros():
        return [jax.device_put(np.zeros((N_CORES * z.shape[0], *z.shape[1:]), z.dtype), sh)
                for z in zero_outs]

    def run_once():
        outs = sharded(*concat_in, *make_zeros())
        return outs

    def to_results(outs):
        res = []
        arrs = [np.asarray(o) for o in outs]
        for c in range(N_CORES):
            res.append({nm: arrs[i].reshape(N_CORES, *out_avals[i].shape)[c]
                        for i, nm in enumerate(out_names)})
        return res

    return dict(sharded=sharded, concat_in=concat_in, make_zeros=make_zeros,
                run_once=run_once, to_results=to_results, set_inputs=set_inputs)


def _struct_key(struct):
    return repr(sorted(struct["Dg"].items())) + repr(sorted(struct["dims"].items()))


def _get_compiled(inputs, dims):
    in_maps, struct = prepare(inputs, dims)
    key = _struct_key(struct)
    if _CACHE.get("key") != key:
        nc = build(struct)
        _CACHE.update(key=key, nc=nc, struct=struct, runner=None)
    return _CACHE["nc"], _CACHE["struct"], in_maps


def _get_runner(nc, in_maps):
    if _CACHE.get("runner") is None:
        _CACHE["runner"] = _make_runner(nc, in_maps)
    else:
        _CACHE["runner"]["set_inputs"](in_maps)
    return _CACHE["runner"]


def run_timed(n=10):
    """Re-execute the compiled kernel n times; returns wall seconds per run."""
    import time
    r = _CACHE["runner"]
    zero_sets = [r["make_zeros"]() for _ in range(n)]
    for z in zero_sets:
        for a in z:
            a.block_until_ready()
    times = []
    for i in range(n):
        t0 = time.perf_counter()
        outs = r["sharded"](*r["concat_in"], *zero_sets[i])
        for o in outs:
            o.block_until_ready()
        times.append(time.perf_counter() - t0)
    return times


def _assemble(results, struct):
    d = struct["dims"]
    g_sh, d_sh = struct["g_sh"], struct["d_sh"]
    h1 = d["h1"]
    xg = np.concatenate([results[c]["gene_out"][:g_sh] for c in range(N_CORES)], axis=0)
    xd = np.concatenate([results[c]["drug_out"][:d_sh] for c in range(N_CORES)], axis=0)
    return np.concatenate([xg, xd], axis=0).astype(np.float32)


def kernel(**inputs):
    nc, struct, in_maps = _get_compiled(inputs, DIMS)
    from concourse.bass_utils import run_bass_kernel_spmd
    res = run_bass_kernel_spmd(nc, in_maps, list(range(N_CORES)))
    return _assemble(res.results, struct)
